# revision 20
# baseline (speedup 1.0000x reference)
"""CrossAttention on 8 TRN2 NeuronCores (tensor-parallel over heads).

Reference computation (B=4, N=2048, DIM=1024, 16 heads, head_dim=64):
    qkv = x @ Wqkv.T + bqkv ; q, k = split(qkv)  (v unused)
    attn = softmax(q @ k.T * scale) ; out = attn @ split_heads(context)
    return merge_heads(out) @ Wout.T + bout

Sharding: core c owns heads {2c, 2c+1}. Each core computes q/k
projections for its heads (full sequence), head-parallel attention with
context slices as values, then an AllToAll re-shards from head-parallel
to row-parallel so the output projection runs locally. Row ownership is
interleaved (core c owns rows [c*256:(c+1)*256] of every batch); the
re-shard is split into two collectives (batches 0-1 and 2-3) so the
first hides under the second half of attention and the second hides
under the output projection of the first batches.

All matmuls run in bf16 (fp32 PSUM accumulation); softmax runs exp on
ScalarE without max-subtraction (scores ~ N(0,1)), with the denominator
produced by an extra all-ones column appended to the value matrix.
The emission order software-pipelines the in-order engine streams:
qk-projection of batch b+1 is sliced into the attention groups of
batch b so ScalarE (the bottleneck) never starves.
"""
import numpy as np
import ml_dtypes

import concourse.bass as bass
import concourse.mybir as mybir
import concourse.tile as tile
from concourse import bacc
from concourse.bass_utils import run_bass_kernel_spmd

BF16 = ml_dtypes.bfloat16
F32 = mybir.dt.float32
BF = mybir.dt.bfloat16

NC = 8            # cores
B = 4             # batch
N = 2048          # sequence
DIM = 1024
NH = 16           # heads total
HD = 64           # head dim
HPC = NH // NC    # heads per core = 2
SCALE = HD ** -0.5
BN = B * N        # 8192 tokens
RPB = N // NC     # rows per (core, batch) after re-shard = 256
KC = DIM // 128   # contraction chunks for projections = 8
NKC = N // 128    # key chunks per batch = 16
CW = HD + 1       # value width incl. ones column = 65


def build(PIPELINE=True, NPHASE=2, MERGEH=False):
    QTAG = 2 if PIPELINE else B
    nc = bacc.Bacc("TRN2", target_bir_lowering=False, debug=False,
                   num_devices=NC)

    xT = nc.dram_tensor("xT", [DIM, BN], BF, kind="ExternalInput")
    wqkT = nc.dram_tensor("wqkT", [DIM, 2 * 128], BF, kind="ExternalInput")
    bqk = nc.dram_tensor("bqk", [2 * 128, 1], F32, kind="ExternalInput")
    ctxa = nc.dram_tensor("ctxa", [B, HPC, 128, NKC * CW], BF,
                          kind="ExternalInput")
    woutT = nc.dram_tensor("woutT", [DIM, DIM], BF, kind="ExternalInput")
    boutb = nc.dram_tensor("boutb", [128, DIM], F32, kind="ExternalInput")
    # out rows: batch-major, 256 rows per batch
    out = nc.dram_tensor("out", [B * RPB, DIM], F32, kind="ExternalOutput")

    # AllToAll bounce buffers, NPHASE collectives each covering B//NPHASE
    # batches; chunk j holds rows [j*256:(j+1)*256] of each covered batch
    bpp = B // NPHASE        # batches per phase
    a2a_in = [nc.dram_tensor(f"a2a_in{p}", [NC, 128, bpp * RPB], BF)
              for p in range(NPHASE)]
    a2a_out = [nc.dram_tensor(f"a2a_out{p}", [NC, 128, bpp * RPB], BF)
               for p in range(NPHASE)]

    rscr = [nc.dram_tensor(f"rscr{i}", [1, 512], F32) for i in range(8)]
    _scr_idx = [0]

    with tile.TileContext(nc) as tc:
        with tc.tile_pool(name="const", bufs=1) as const, \
             tc.tile_pool(name="qk", bufs=1) as qkpool, \
             tc.tile_pool(name="xt", bufs=8 if not PIPELINE else 10) as xtpool, \
             tc.tile_pool(name="pt", bufs=2) as ptpool, \
             tc.tile_pool(name="r1", bufs=4) as r1pool, \
             tc.tile_pool(name="rb", bufs=4) as rbpool, \
             tc.tile_pool(name="ho", bufs=4) as hopool, \
             tc.tile_pool(name="sl", bufs=16) as slpool, \
             tc.tile_pool(name="ob", bufs=4) as obpool, \
             tc.tile_pool(name="pc", bufs=3) as pcpool, \
             tc.tile_pool(name="pss", bufs=2, space="PSUM") as pss_pool, \
             tc.tile_pool(name="psm", bufs=4, space="PSUM") as psm_pool:

            # ---- small constants needed up front ----
            wqk_sb = []
            for kc in range(KC):
                t = const.tile([128, 256], BF, tag=f"wqk{kc}")
                nc.sync.dma_start(out=t[:], in_=wqkT[kc * 128:(kc + 1) * 128, :])
                wqk_sb.append(t)
            bq_sb = []
            for fb in range(2):
                t = const.tile([128, 1], F32, tag=f"bq{fb}")
                nc.sync.dma_start(out=t[:], in_=bqk[fb * 128:(fb + 1) * 128, :])
                bq_sb.append(t)

            wout_sb = []
            bout_sb = const.tile([128, DIM], F32, tag="bout")
            ctx_sb = {}
            qk_tiles = {}
            xt_tiles = {}

            def load_out_consts():
                for fc in range(KC):
                    t = const.tile([128, DIM], BF, tag=f"wout{fc}",
                                   name=f"wout{fc}")
                    nc.sync.dma_start(
                        out=t[:], in_=woutT[fc * 128:(fc + 1) * 128, :])
                    wout_sb.append(t)
                nc.sync.dma_start(out=bout_sb[:], in_=boutb[:])

            def load_ctx(b):
                for h in range(HPC):
                    t = const.tile([128, NKC * CW], BF, tag=f"ctx{b}{h}",
                                   name=f"ctx{b}_{h}")
                    nc.sync.dma_start(out=t[:], in_=ctxa[b, h, :, :])
                    ctx_sb[b, h] = t

            def prefetch_x(b):
                """Issue the xT DMAs and allocate q/k tiles for batch b."""
                qT = qkpool.tile([128, N], BF, tag=f"qT{b % QTAG}", name=f"qT{b}")
                kT = qkpool.tile([128, N], BF, tag=f"kT{b % QTAG}", name=f"kT{b}")
                qk_tiles[b] = (qT, kT)
                xts = []
                for kc in range(KC):
                    xt = xtpool.tile([128, N], BF, tag="xt",
                                     name=f"xtb{b}_{kc}")
                    nc.sync.dma_start(
                        out=xt[:], in_=xT[kc * 128:(kc + 1) * 128,
                                          b * N:(b + 1) * N])
                    xts.append(xt)
                xt_tiles[b] = xts

            def qkproj_slice(b, t):
                """Project token chunk t (512 tokens) of batch b."""
                qT, kT = qk_tiles[b]
                xts = xt_tiles[b]
                for fb, dst in ((1, kT), (0, qT)):
                    ps = psm_pool.tile([128, 512], F32, tag="psm",
                                       name=f"psq{b}_{t}_{fb}")
                    for kc in range(KC):
                        nc.tensor.matmul(
                            ps[:], wqk_sb[kc][:, fb * 128:(fb + 1) * 128],
                            xts[kc][:, t * 512:(t + 1) * 512],
                            start=(kc == 0), stop=(kc == KC - 1))
                    nc.vector.tensor_scalar_add(
                        dst[:, t * 512:(t + 1) * 512], ps[:], bq_sb[fb][:])

            def attention_group(b, h, qg):
                """Scores+softmax+values for one (head, 1024-query) group."""
                qT, kT = qk_tiles[b]
                hp = h * HD
                q0 = qg * 1024
                pt = ptpool.tile([128, NKC * 1024], BF, tag="pt",
                                 name=f"pt{b}_{h}_{qg}")
                for kc in range(NKC):
                    ps = pss_pool.tile([128, 1024], F32, tag="pss",
                                       name=f"pss{b}{h}{qg}{kc}")
                    for hf in range(2):
                        nc.tensor.matmul(
                            ps[:, hf * 512:(hf + 1) * 512],
                            kT[hp:hp + HD, kc * 128:(kc + 1) * 128],
                            qT[hp:hp + HD, q0 + hf * 512:q0 + (hf + 1) * 512],
                            start=True, stop=True)
                    nc.scalar.activation(
                        pt[:, kc * 1024:(kc + 1) * 1024], ps[:],
                        mybir.ActivationFunctionType.Exp, scale=SCALE)
                for qc in range(2):  # 512-query chunks
                    pav = psm_pool.tile([CW, 512], F32, tag="psm",
                                        name=f"pav{b}{h}{qg}{qc}")
                    for kc in range(NKC):
                        nc.tensor.matmul(
                            pav[:], ctx_sb[b, h][:, kc * CW:(kc + 1) * CW],
                            pt[:, kc * 1024 + qc * 512:
                               kc * 1024 + (qc + 1) * 512],
                            start=(kc == 0), stop=(kc == NKC - 1))
                    r1 = r1pool.tile([1, 512], F32, tag="r1",
                                     name=f"r1{b}{h}{qg}{qc}")
                    nc.vector.reciprocal(r1[:], pav[HD:CW, :])
                    # broadcast partition 0 -> 64 via a DRAM round-trip so
                    # gpsimd stays free to run collectives asynchronously
                    scr = rscr[_scr_idx[0] % 8]; _scr_idx[0] += 1
                    nc.sync.dma_start(out=scr[:], in_=r1[:])
                    rb = rbpool.tile([HD, 512], F32, tag="rb",
                                     name=f"rb{b}{h}{qg}{qc}")
                    nc.sync.dma_start(out=rb[:],
                                      in_=scr[:].broadcast_to([HD, 512]))
                    ho = hopool.tile([HD, 512], BF, tag="ho",
                                     name=f"ho{b}{h}{qg}{qc}")
                    nc.vector.tensor_tensor(
                        out=ho[:], in0=pav[0:HD, :], in1=rb[:],
                        op=mybir.AluOpType.mult)
                    # queries qq0..qq0+512 span two 256-row chunks
                    qq0 = q0 + qc * 512
                    for half in range(2):
                        j = (qq0 + half * 256) // RPB
                        o = (b % bpp) * RPB
                        nc.sync.dma_start(
                            out=a2a_in[b // bpp][j, h * HD:(h + 1) * HD,
                                                 o:o + RPB],
                            in_=ho[:, half * 256:(half + 1) * 256])

            def attention_pair(b, qg):
                """Both heads' scores+softmax+values for 512 queries.

                The two heads' score matmuls contract over disjoint
                row-groups of the PE array (partitions 0-63 / 64-127) and
                write disjoint PSUM banks, so they run concurrently.
                """
                qT, kT = qk_tiles[b]
                q0 = qg * 512
                pt = ptpool.tile([128, NKC * 1024], BF, tag="pt",
                                 name=f"ptp{b}_{qg}")
                for kc in range(NKC):
                    ps = pss_pool.tile([128, 1024], F32, tag="pss",
                                       name=f"pssp{b}{qg}{kc}")
                    for h in range(HPC):
                        nc.tensor.matmul(
                            ps[:, h * 512:(h + 1) * 512],
                            kT[h * HD:(h + 1) * HD, kc * 128:(kc + 1) * 128],
                            qT[h * HD:(h + 1) * HD, q0:q0 + 512],
                            start=True, stop=True,
                            tile_position=(h * HD, 0))
                    nc.scalar.activation(
                        pt[:, kc * 1024:(kc + 1) * 1024], ps[:],
                        mybir.ActivationFunctionType.Exp, scale=SCALE)
                for h in range(HPC):
                    pav = psm_pool.tile([CW, 512], F32, tag="psm",
                                        name=f"pavp{b}{qg}{h}")
                    for kc in range(NKC):
                        nc.tensor.matmul(
                            pav[:], ctx_sb[b, h][:, kc * CW:(kc + 1) * CW],
                            pt[:, kc * 1024 + h * 512:
                               kc * 1024 + (h + 1) * 512],
                            start=(kc == 0), stop=(kc == NKC - 1))
                    # evict PSUM immediately so the accumulator slot
                    # frees before the (long-latency) normalize chain
                    pc = pcpool.tile([CW, 512], F32, tag="pc",
                                     name=f"pcp{b}{qg}{h}")
                    nc.vector.tensor_copy(pc[:], pav[:])
                    r1 = r1pool.tile([1, 512], F32, tag="r1",
                                     name=f"r1p{b}{qg}{h}")
                    nc.vector.reciprocal(r1[:], pc[HD:CW, :])
                    scr = rscr[_scr_idx[0] % 8]; _scr_idx[0] += 1
                    nc.sync.dma_start(out=scr[:], in_=r1[:])
                    rb = rbpool.tile([HD, 512], F32, tag="rb",
                                     name=f"rbp{b}{qg}{h}")
                    nc.sync.dma_start(out=rb[:],
                                      in_=scr[:].broadcast_to([HD, 512]))
                    ho = hopool.tile([HD, 512], BF, tag="ho",
                                     name=f"hop{b}{qg}{h}")
                    nc.vector.tensor_tensor(
                        out=ho[:], in0=pc[0:HD, :], in1=rb[:],
                        op=mybir.AluOpType.mult)
                    for half in range(2):
                        j = (q0 + half * 256) // RPB
                        o = (b % bpp) * RPB
                        nc.sync.dma_start(
                            out=a2a_in[b // bpp][j, h * HD:(h + 1) * HD,
                                                 o:o + RPB],
                            in_=ho[:, half * 256:(half + 1) * 256])

            def reshard(p):
                nc.gpsimd.collective_compute(
                    "AllToAll", mybir.AluOpType.bypass,
                    replica_groups=[list(range(NC))],
                    ins=[a2a_in[p].ap().opt()], outs=[a2a_out[p].ap().opt()])

            def outproj(b):
                """Output projection for my 256 rows of batch b."""
                p, o = b // bpp, (b % bpp) * RPB
                for rc in range(RPB // 128):
                    sls = []
                    for fc in range(KC):
                        sl = slpool.tile([128, 128], BF, tag="sl",
                                         name=f"sl{b}_{rc}_{fc}")
                        nc.sync.dma_start(
                            out=sl[:],
                            in_=a2a_out[p][fc, :,
                                           o + rc * 128:o + (rc + 1) * 128])
                        sls.append(sl)
                    pso = [psm_pool.tile([128, 512], F32, tag="psm",
                                         name=f"pso{b}_{rc}_{i}")
                           for i in range(2)]
                    for fc in range(KC):
                        for n in range(2):
                            nc.tensor.matmul(
                                pso[n][:], sls[fc][:],
                                wout_sb[fc][:, n * 512:(n + 1) * 512],
                                start=(fc == 0), stop=(fc == KC - 1))
                    for n in range(2):
                        ob = obpool.tile([128, 512], F32, tag="ob",
                                         name=f"ob{b}_{rc}_{n}")
                        nc.vector.tensor_tensor(
                            out=ob[:], in0=pso[n][:],
                            in1=bout_sb[:, n * 512:(n + 1) * 512],
                            op=mybir.AluOpType.add)
                        nc.sync.dma_start(
                            out=out[b * RPB + rc * 128:
                                    b * RPB + (rc + 1) * 128,
                                    n * 512:(n + 1) * 512],
                            in_=ob[:])

            if PIPELINE:
                # software-pipelined emission
                prefetch_x(0)
                load_ctx(0)
                for t in range(4):
                    qkproj_slice(0, t)
                for b in range(B):
                    if b + 1 < B:
                        prefetch_x(b + 1)
                        load_ctx(b + 1)
                    for g, (h, qg) in enumerate(
                            ((0, 0), (0, 1), (1, 0), (1, 1))):
                        attention_group(b, h, qg)
                        if b + 1 < B:
                            qkproj_slice(b + 1, g)
                        elif g == 1:
                            load_out_consts()
                    if (b + 1) % bpp == 0:
                        reshard(b // bpp)
                for b in range(B):
                    outproj(b)
            else:
                # monolithic phases (v1-style), with the qk projection of
                # later batches staggered after earlier batches' attention
                # so the cold-clock ramp only fronts two batches of work
                for b in range(2):
                    prefetch_x(b)
                    load_ctx(b)
                    for t in range(4):
                        qkproj_slice(b, t)
                load_out_consts()
                for b in range(B):
                    if MERGEH:
                        if b + 2 < B:
                            prefetch_x(b + 2)
                            load_ctx(b + 2)
                        for qg in range(4):
                            attention_pair(b, qg)
                            if b + 2 < B:
                                qkproj_slice(b + 2, qg)
                            # fill PE slack in the last batch with the
                            # output projection of the already-resharded
                            # first phase
                        if b == B - 1 and NPHASE > 1:
                            for bb in range(bpp):
                                outproj(bb)
                    else:
                        if b + 2 < B:
                            prefetch_x(b + 2)
                            load_ctx(b + 2)
                            for t in range(4):
                                qkproj_slice(b + 2, t)
                        for h, qg in ((0, 0), (0, 1), (1, 0), (1, 1)):
                            attention_group(b, h, qg)
                    if (b + 1) % bpp == 0:
                        reshard(b // bpp)
                for b in range(bpp if (MERGEH and NPHASE > 1) else 0, B):
                    outproj(b)
    nc.compile()
    return nc


def prep_inputs(x, context, Wqkv, bqkv, Wout, bout):
    """Host-side sharding: returns in_maps for the 8 cores."""
    x = np.asarray(x, np.float32)
    context = np.asarray(context, np.float32)
    Wqkv = np.asarray(Wqkv, np.float32)
    bqkv = np.asarray(bqkv, np.float32)
    Wout = np.asarray(Wout, np.float32)
    bout = np.asarray(bout, np.float32)

    xT = np.ascontiguousarray(x.reshape(BN, DIM).T).astype(BF16)
    woutT = np.ascontiguousarray(Wout.T).astype(BF16)
    boutb = np.broadcast_to(bout, (128, DIM)).astype(np.float32).copy()

    in_maps = []
    for c in range(NC):
        h0 = c * HPC
        # feature order: [q_h0 | q_h1] then [k_h0 | k_h1]
        wq = Wqkv[h0 * HD:(h0 + HPC) * HD]
        wk = Wqkv[DIM + h0 * HD:DIM + (h0 + HPC) * HD]
        wqkT = np.ascontiguousarray(
            np.concatenate([wq, wk], axis=0).T).astype(BF16)
        bq = np.concatenate([bqkv[h0 * HD:(h0 + HPC) * HD],
                             bqkv[DIM + h0 * HD:DIM + (h0 + HPC) * HD]])
        bq = bq.reshape(2 * 128, 1).astype(np.float32)
        ctxa = np.ones((B, HPC, 128, NKC, CW), np.float32)
        for h in range(HPC):
            g = h0 + h
            arr = context[:, :, g * HD:(g + 1) * HD].reshape(B, NKC, 128, HD)
            ctxa[:, h, :, :, :HD] = arr.transpose(0, 2, 1, 3)
        in_maps.append({
            "xT": xT,
            "wqkT": wqkT,
            "bqk": bq,
            "ctxa": ctxa.reshape(B, HPC, 128, NKC * CW).astype(BF16),
            "woutT": woutT,
            "boutb": boutb,
        })
    return in_maps


_NC_CACHE = None


import os


def _get_nc():
    global _NC_CACHE
    if _NC_CACHE is None:
        _NC_CACHE = build(
            PIPELINE=os.environ.get("K_PIPELINE", "0") == "1",
            NPHASE=int(os.environ.get("K_NPHASE", "2")),
            MERGEH=os.environ.get("K_MERGEH", "1") == "1")
    return _NC_CACHE


def run(in_maps, trace=False):
    nc = _get_nc()
    res = run_bass_kernel_spmd(nc, in_maps, core_ids=list(range(NC)),
                               trace=trace)
    # core c's out = [B*256, DIM]: rows [c*256:(c+1)*256] of each batch
    full = np.empty((B, N, DIM), np.float32)
    for c in range(NC):
        o = np.asarray(res.results[c]["out"]).reshape(B, RPB, DIM)
        full[:, c * RPB:(c + 1) * RPB, :] = o
    return full, res


def kernel(x, context, Wqkv, bqkv, Wout, bout):
    in_maps = prep_inputs(x, context, Wqkv, bqkv, Wout, bout)
    out, _ = run(in_maps, trace=False)
    return out


# revision 22
# speedup vs baseline: 1.0757x; 1.0757x over previous
"""CrossAttention on 8 TRN2 NeuronCores (tensor-parallel over heads).

Reference computation (B=4, N=2048, DIM=1024, 16 heads, head_dim=64):
    qkv = x @ Wqkv.T + bqkv ; q, k = split(qkv)  (v unused)
    attn = softmax(q @ k.T * scale) ; out = attn @ split_heads(context)
    return merge_heads(out) @ Wout.T + bout

Sharding: core c owns heads {2c, 2c+1}. Each core computes q/k
projections for its heads (full sequence), head-parallel attention with
context slices as values, then an AllToAll re-shards from head-parallel
to row-parallel so the output projection runs locally. Row ownership is
interleaved (core c owns rows [c*256:(c+1)*256] of every batch); the
re-shard is split into two collectives (batches 0-1 and 2-3) so the
first hides under the second half of attention and the second hides
under the output projection of the first batches.

All matmuls run in bf16 (fp32 PSUM accumulation); softmax runs exp on
ScalarE without max-subtraction (scores ~ N(0,1)), with the denominator
produced by an extra all-ones column appended to the value matrix.
The emission order software-pipelines the in-order engine streams:
qk-projection of batch b+1 is sliced into the attention groups of
batch b so ScalarE (the bottleneck) never starves.
"""
import numpy as np
import ml_dtypes

import concourse.bass as bass
import concourse.mybir as mybir
import concourse.tile as tile
from concourse import bacc
from concourse.bass_utils import run_bass_kernel_spmd

BF16 = ml_dtypes.bfloat16
F32 = mybir.dt.float32
BF = mybir.dt.bfloat16

NC = 8            # cores
B = 4             # batch
N = 2048          # sequence
DIM = 1024
NH = 16           # heads total
HD = 64           # head dim
HPC = NH // NC    # heads per core = 2
SCALE = HD ** -0.5
BN = B * N        # 8192 tokens
RPB = N // NC     # rows per (core, batch) after re-shard = 256
KC = DIM // 128   # contraction chunks for projections = 8
NKC = N // 128    # key chunks per batch = 16
CW = HD + 1       # value width incl. ones column = 65


def build(PIPELINE=True, NPHASE=2, MERGEH=False):
    QTAG = 2 if PIPELINE else B
    nc = bacc.Bacc("TRN2", target_bir_lowering=False, debug=False,
                   num_devices=NC)

    xT = nc.dram_tensor("xT", [DIM, BN], BF, kind="ExternalInput")
    wqkT = nc.dram_tensor("wqkT", [DIM, 2 * 128], BF, kind="ExternalInput")
    bqk = nc.dram_tensor("bqk", [2 * 128, 1], F32, kind="ExternalInput")
    ctxa = nc.dram_tensor("ctxa", [B, HPC, 128, NKC * CW], BF,
                          kind="ExternalInput")
    woutT = nc.dram_tensor("woutT", [DIM, DIM], BF, kind="ExternalInput")
    boutb = nc.dram_tensor("boutb", [128, DIM], F32, kind="ExternalInput")
    # out rows: batch-major, 256 rows per batch
    out = nc.dram_tensor("out", [B * RPB, DIM], F32, kind="ExternalOutput")

    # AllToAll bounce buffers, NPHASE collectives each covering B//NPHASE
    # batches; chunk j holds rows [j*256:(j+1)*256] of each covered batch
    bpp = B // NPHASE        # batches per phase
    a2a_in = [nc.dram_tensor(f"a2a_in{p}", [NC, 128, bpp * RPB], BF)
              for p in range(NPHASE)]
    a2a_out = [nc.dram_tensor(f"a2a_out{p}", [NC, 128, bpp * RPB], BF)
               for p in range(NPHASE)]

    rscr = [nc.dram_tensor(f"rscr{i}", [1, 512], F32) for i in range(8)]
    _scr_idx = [0]

    with tile.TileContext(nc) as tc:
        with tc.tile_pool(name="const", bufs=1) as const, \
             tc.tile_pool(name="qk", bufs=1) as qkpool, \
             tc.tile_pool(name="xt", bufs=8 if not PIPELINE else 10) as xtpool, \
             tc.tile_pool(name="pt", bufs=2) as ptpool, \
             tc.tile_pool(name="r1", bufs=4) as r1pool, \
             tc.tile_pool(name="rb", bufs=4) as rbpool, \
             tc.tile_pool(name="ho", bufs=4) as hopool, \
             tc.tile_pool(name="sl", bufs=16) as slpool, \
             tc.tile_pool(name="ob", bufs=4) as obpool, \
             tc.tile_pool(name="pc", bufs=3) as pcpool, \
             tc.tile_pool(name="pss", bufs=2, space="PSUM") as pss_pool, \
             tc.tile_pool(name="psm", bufs=4, space="PSUM") as psm_pool:

            # ---- small constants needed up front ----
            wqk_sb = []
            for kc in range(KC):
                t = const.tile([128, 256], BF, tag=f"wqk{kc}")
                nc.sync.dma_start(out=t[:], in_=wqkT[kc * 128:(kc + 1) * 128, :])
                wqk_sb.append(t)
            bq_sb = []
            for fb in range(2):
                t = const.tile([128, 1], F32, tag=f"bq{fb}")
                nc.sync.dma_start(out=t[:], in_=bqk[fb * 128:(fb + 1) * 128, :])
                bq_sb.append(t)

            wout_sb = []
            bout_sb = const.tile([128, DIM], F32, tag="bout")
            ctx_sb = {}
            qk_tiles = {}
            xt_tiles = {}

            def load_out_consts():
                for fc in range(KC):
                    t = const.tile([128, DIM], BF, tag=f"wout{fc}",
                                   name=f"wout{fc}")
                    nc.sync.dma_start(
                        out=t[:], in_=woutT[fc * 128:(fc + 1) * 128, :])
                    wout_sb.append(t)
                nc.sync.dma_start(out=bout_sb[:], in_=boutb[:])

            def load_ctx(b):
                for h in range(HPC):
                    t = const.tile([128, NKC * CW], BF, tag=f"ctx{b}{h}",
                                   name=f"ctx{b}_{h}")
                    nc.sync.dma_start(out=t[:], in_=ctxa[b, h, :, :])
                    ctx_sb[b, h] = t

            def prefetch_x(b):
                """Issue the xT DMAs and allocate q/k tiles for batch b."""
                qT = qkpool.tile([128, N], BF, tag=f"qT{b % QTAG}", name=f"qT{b}")
                kT = qkpool.tile([128, N], BF, tag=f"kT{b % QTAG}", name=f"kT{b}")
                qk_tiles[b] = (qT, kT)
                xts = []
                for kc in range(KC):
                    xt = xtpool.tile([128, N], BF, tag="xt",
                                     name=f"xtb{b}_{kc}")
                    nc.sync.dma_start(
                        out=xt[:], in_=xT[kc * 128:(kc + 1) * 128,
                                          b * N:(b + 1) * N])
                    xts.append(xt)
                xt_tiles[b] = xts

            def qkproj_slice(b, t):
                """Project token chunk t (512 tokens) of batch b."""
                qT, kT = qk_tiles[b]
                xts = xt_tiles[b]
                for fb, dst in ((1, kT), (0, qT)):
                    ps = psm_pool.tile([128, 512], F32, tag="psm",
                                       name=f"psq{b}_{t}_{fb}")
                    for kc in range(KC):
                        nc.tensor.matmul(
                            ps[:], wqk_sb[kc][:, fb * 128:(fb + 1) * 128],
                            xts[kc][:, t * 512:(t + 1) * 512],
                            start=(kc == 0), stop=(kc == KC - 1))
                    nc.vector.tensor_scalar_add(
                        dst[:, t * 512:(t + 1) * 512], ps[:], bq_sb[fb][:])

            def attention_group(b, h, qg):
                """Scores+softmax+values for one (head, 1024-query) group."""
                qT, kT = qk_tiles[b]
                hp = h * HD
                q0 = qg * 1024
                pt = ptpool.tile([128, NKC * 1024], BF, tag="pt",
                                 name=f"pt{b}_{h}_{qg}")
                for kc in range(NKC):
                    ps = pss_pool.tile([128, 1024], F32, tag="pss",
                                       name=f"pss{b}{h}{qg}{kc}")
                    for hf in range(2):
                        nc.tensor.matmul(
                            ps[:, hf * 512:(hf + 1) * 512],
                            kT[hp:hp + HD, kc * 128:(kc + 1) * 128],
                            qT[hp:hp + HD, q0 + hf * 512:q0 + (hf + 1) * 512],
                            start=True, stop=True)
                    nc.scalar.activation(
                        pt[:, kc * 1024:(kc + 1) * 1024], ps[:],
                        mybir.ActivationFunctionType.Exp, scale=SCALE)
                for qc in range(2):  # 512-query chunks
                    pav = psm_pool.tile([CW, 512], F32, tag="psm",
                                        name=f"pav{b}{h}{qg}{qc}")
                    for kc in range(NKC):
                        nc.tensor.matmul(
                            pav[:], ctx_sb[b, h][:, kc * CW:(kc + 1) * CW],
                            pt[:, kc * 1024 + qc * 512:
                               kc * 1024 + (qc + 1) * 512],
                            start=(kc == 0), stop=(kc == NKC - 1))
                    r1 = r1pool.tile([1, 512], F32, tag="r1",
                                     name=f"r1{b}{h}{qg}{qc}")
                    nc.vector.reciprocal(r1[:], pav[HD:CW, :])
                    # broadcast partition 0 -> 64 via a DRAM round-trip so
                    # gpsimd stays free to run collectives asynchronously
                    scr = rscr[_scr_idx[0] % 8]; _scr_idx[0] += 1
                    nc.sync.dma_start(out=scr[:], in_=r1[:])
                    rb = rbpool.tile([HD, 512], F32, tag="rb",
                                     name=f"rb{b}{h}{qg}{qc}")
                    nc.sync.dma_start(out=rb[:],
                                      in_=scr[:].broadcast_to([HD, 512]))
                    ho = hopool.tile([HD, 512], BF, tag="ho",
                                     name=f"ho{b}{h}{qg}{qc}")
                    nc.vector.tensor_tensor(
                        out=ho[:], in0=pav[0:HD, :], in1=rb[:],
                        op=mybir.AluOpType.mult)
                    # queries qq0..qq0+512 span two 256-row chunks
                    qq0 = q0 + qc * 512
                    for half in range(2):
                        j = (qq0 + half * 256) // RPB
                        o = (b % bpp) * RPB
                        nc.sync.dma_start(
                            out=a2a_in[b // bpp][j, h * HD:(h + 1) * HD,
                                                 o:o + RPB],
                            in_=ho[:, half * 256:(half + 1) * 256])

            def attention_pair(b, qg):
                """Both heads' scores+softmax+values for 512 queries.

                The two heads' score matmuls contract over disjoint
                row-groups of the PE array (partitions 0-63 / 64-127) and
                write disjoint PSUM banks, so they run concurrently.
                """
                qT, kT = qk_tiles[b]
                q0 = qg * 512
                pt = ptpool.tile([128, NKC * 1024], BF, tag="pt",
                                 name=f"ptp{b}_{qg}")
                for kc in range(NKC):
                    ps = pss_pool.tile([128, 1024], F32, tag="pss",
                                       name=f"pssp{b}{qg}{kc}")
                    for h in range(HPC):
                        nc.tensor.matmul(
                            ps[:, h * 512:(h + 1) * 512],
                            kT[h * HD:(h + 1) * HD, kc * 128:(kc + 1) * 128],
                            qT[h * HD:(h + 1) * HD, q0:q0 + 512],
                            start=True, stop=True,
                            tile_position=(h * HD, 0))
                    nc.scalar.activation(
                        pt[:, kc * 1024:(kc + 1) * 1024], ps[:],
                        mybir.ActivationFunctionType.Exp, scale=SCALE)
                for h in range(HPC):
                    pav = psm_pool.tile([CW, 512], F32, tag="psm",
                                        name=f"pavp{b}{qg}{h}")
                    for kc in range(NKC):
                        nc.tensor.matmul(
                            pav[:], ctx_sb[b, h][:, kc * CW:(kc + 1) * CW],
                            pt[:, kc * 1024 + h * 512:
                               kc * 1024 + (h + 1) * 512],
                            start=(kc == 0), stop=(kc == NKC - 1))
                    # evict PSUM immediately so the accumulator slot
                    # frees before the (long-latency) normalize chain
                    pc = pcpool.tile([CW, 512], F32, tag="pc",
                                     name=f"pcp{b}{qg}{h}")
                    nc.vector.tensor_copy(pc[:], pav[:])
                    r1 = r1pool.tile([1, 512], F32, tag="r1",
                                     name=f"r1p{b}{qg}{h}")
                    nc.vector.reciprocal(r1[:], pc[HD:CW, :])
                    scr = rscr[_scr_idx[0] % 8]; _scr_idx[0] += 1
                    nc.sync.dma_start(out=scr[:], in_=r1[:])
                    rb = rbpool.tile([HD, 512], F32, tag="rb",
                                     name=f"rbp{b}{qg}{h}")
                    nc.sync.dma_start(out=rb[:],
                                      in_=scr[:].broadcast_to([HD, 512]))
                    ho = hopool.tile([HD, 512], BF, tag="ho",
                                     name=f"hop{b}{qg}{h}")
                    nc.vector.tensor_tensor(
                        out=ho[:], in0=pc[0:HD, :], in1=rb[:],
                        op=mybir.AluOpType.mult)
                    for half in range(2):
                        j = (q0 + half * 256) // RPB
                        o = (b % bpp) * RPB
                        nc.sync.dma_start(
                            out=a2a_in[b // bpp][j, h * HD:(h + 1) * HD,
                                                 o:o + RPB],
                            in_=ho[:, half * 256:(half + 1) * 256])

            def reshard(p):
                nc.gpsimd.collective_compute(
                    "AllToAll", mybir.AluOpType.bypass,
                    replica_groups=[list(range(NC))],
                    ins=[a2a_in[p].ap().opt()], outs=[a2a_out[p].ap().opt()])

            def outproj(b):
                """Output projection for my 256 rows of batch b."""
                p, o = b // bpp, (b % bpp) * RPB
                for rc in range(RPB // 128):
                    sls = []
                    for fc in range(KC):
                        sl = slpool.tile([128, 128], BF, tag="sl",
                                         name=f"sl{b}_{rc}_{fc}")
                        nc.sync.dma_start(
                            out=sl[:],
                            in_=a2a_out[p][fc, :,
                                           o + rc * 128:o + (rc + 1) * 128])
                        sls.append(sl)
                    pso = [psm_pool.tile([128, 512], F32, tag="psm",
                                         name=f"pso{b}_{rc}_{i}")
                           for i in range(2)]
                    for fc in range(KC):
                        for n in range(2):
                            nc.tensor.matmul(
                                pso[n][:], sls[fc][:],
                                wout_sb[fc][:, n * 512:(n + 1) * 512],
                                start=(fc == 0), stop=(fc == KC - 1))
                    for n in range(2):
                        ob = obpool.tile([128, 512], F32, tag="ob",
                                         name=f"ob{b}_{rc}_{n}")
                        nc.vector.tensor_tensor(
                            out=ob[:], in0=pso[n][:],
                            in1=bout_sb[:, n * 512:(n + 1) * 512],
                            op=mybir.AluOpType.add)
                        nc.sync.dma_start(
                            out=out[b * RPB + rc * 128:
                                    b * RPB + (rc + 1) * 128,
                                    n * 512:(n + 1) * 512],
                            in_=ob[:])

            if PIPELINE:
                # software-pipelined emission
                prefetch_x(0)
                load_ctx(0)
                for t in range(4):
                    qkproj_slice(0, t)
                for b in range(B):
                    if b + 1 < B:
                        prefetch_x(b + 1)
                        load_ctx(b + 1)
                    for g, (h, qg) in enumerate(
                            ((0, 0), (0, 1), (1, 0), (1, 1))):
                        attention_group(b, h, qg)
                        if b + 1 < B:
                            qkproj_slice(b + 1, g)
                        elif g == 1:
                            load_out_consts()
                    if (b + 1) % bpp == 0:
                        reshard(b // bpp)
                for b in range(B):
                    outproj(b)
            else:
                # monolithic phases (v1-style), with the qk projection of
                # later batches staggered after earlier batches' attention
                # so the cold-clock ramp only fronts two batches of work
                for b in range(2):
                    prefetch_x(b)
                    load_ctx(b)
                    for t in range(4):
                        qkproj_slice(b, t)
                load_out_consts()
                for b in range(B):
                    if MERGEH:
                        if b + 2 < B:
                            prefetch_x(b + 2)
                            load_ctx(b + 2)
                            for t in range(4):
                                qkproj_slice(b + 2, t)
                        for qg in range(4):
                            attention_pair(b, qg)
                            # fill PE slack in the last batch with the
                            # output projection of the already-resharded
                            # first phase
                        if b == B - 1 and NPHASE > 1:
                            for bb in range(bpp):
                                outproj(bb)
                    else:
                        if b + 2 < B:
                            prefetch_x(b + 2)
                            load_ctx(b + 2)
                            for t in range(4):
                                qkproj_slice(b + 2, t)
                        for h, qg in ((0, 0), (0, 1), (1, 0), (1, 1)):
                            attention_group(b, h, qg)
                    if (b + 1) % bpp == 0:
                        reshard(b // bpp)
                for b in range(bpp if (MERGEH and NPHASE > 1) else 0, B):
                    outproj(b)
    nc.compile()
    return nc


def prep_inputs(x, context, Wqkv, bqkv, Wout, bout):
    """Host-side sharding: returns in_maps for the 8 cores."""
    x = np.asarray(x, np.float32)
    context = np.asarray(context, np.float32)
    Wqkv = np.asarray(Wqkv, np.float32)
    bqkv = np.asarray(bqkv, np.float32)
    Wout = np.asarray(Wout, np.float32)
    bout = np.asarray(bout, np.float32)

    xT = np.ascontiguousarray(x.reshape(BN, DIM).T).astype(BF16)
    woutT = np.ascontiguousarray(Wout.T).astype(BF16)
    boutb = np.broadcast_to(bout, (128, DIM)).astype(np.float32).copy()

    in_maps = []
    for c in range(NC):
        h0 = c * HPC
        # feature order: [q_h0 | q_h1] then [k_h0 | k_h1]
        wq = Wqkv[h0 * HD:(h0 + HPC) * HD]
        wk = Wqkv[DIM + h0 * HD:DIM + (h0 + HPC) * HD]
        wqkT = np.ascontiguousarray(
            np.concatenate([wq, wk], axis=0).T).astype(BF16)
        bq = np.concatenate([bqkv[h0 * HD:(h0 + HPC) * HD],
                             bqkv[DIM + h0 * HD:DIM + (h0 + HPC) * HD]])
        bq = bq.reshape(2 * 128, 1).astype(np.float32)
        ctxa = np.ones((B, HPC, 128, NKC, CW), np.float32)
        for h in range(HPC):
            g = h0 + h
            arr = context[:, :, g * HD:(g + 1) * HD].reshape(B, NKC, 128, HD)
            ctxa[:, h, :, :, :HD] = arr.transpose(0, 2, 1, 3)
        in_maps.append({
            "xT": xT,
            "wqkT": wqkT,
            "bqk": bq,
            "ctxa": ctxa.reshape(B, HPC, 128, NKC * CW).astype(BF16),
            "woutT": woutT,
            "boutb": boutb,
        })
    return in_maps


_NC_CACHE = None


import os


def _get_nc():
    global _NC_CACHE
    if _NC_CACHE is None:
        _NC_CACHE = build(
            PIPELINE=os.environ.get("K_PIPELINE", "0") == "1",
            NPHASE=int(os.environ.get("K_NPHASE", "2")),
            MERGEH=os.environ.get("K_MERGEH", "1") == "1")
    return _NC_CACHE


def run(in_maps, trace=False):
    nc = _get_nc()
    res = run_bass_kernel_spmd(nc, in_maps, core_ids=list(range(NC)),
                               trace=trace)
    # core c's out = [B*256, DIM]: rows [c*256:(c+1)*256] of each batch
    full = np.empty((B, N, DIM), np.float32)
    for c in range(NC):
        o = np.asarray(res.results[c]["out"]).reshape(B, RPB, DIM)
        full[:, c * RPB:(c + 1) * RPB, :] = o
    return full, res


def kernel(x, context, Wqkv, bqkv, Wout, bout):
    in_maps = prep_inputs(x, context, Wqkv, bqkv, Wout, bout)
    out, _ = run(in_maps, trace=False)
    return out


# revision 24
# speedup vs baseline: 1.0763x; 1.0006x over previous
"""CrossAttention on 8 TRN2 NeuronCores (tensor-parallel over heads).

Reference computation (B=4, N=2048, DIM=1024, 16 heads, head_dim=64):
    qkv = x @ Wqkv.T + bqkv ; q, k = split(qkv)  (v unused)
    attn = softmax(q @ k.T * scale) ; out = attn @ split_heads(context)
    return merge_heads(out) @ Wout.T + bout

Sharding: core c owns heads {2c, 2c+1}. Each core computes q/k
projections for its heads (full sequence), head-parallel attention with
context slices as values, then an AllToAll re-shards from head-parallel
to row-parallel so the output projection runs locally. Row ownership is
interleaved (core c owns rows [c*256:(c+1)*256] of every batch); the
re-shard is split into two collectives (batches 0-1 and 2-3) so the
first hides under the second half of attention and the second hides
under the output projection of the first batches.

All matmuls run in bf16 (fp32 PSUM accumulation); softmax runs exp on
ScalarE without max-subtraction (scores ~ N(0,1)), with the denominator
produced by an extra all-ones column appended to the value matrix.
The emission order software-pipelines the in-order engine streams:
qk-projection of batch b+1 is sliced into the attention groups of
batch b so ScalarE (the bottleneck) never starves.
"""
import numpy as np
import ml_dtypes

import concourse.bass as bass
import concourse.mybir as mybir
import concourse.tile as tile
from concourse import bacc
from concourse.bass_utils import run_bass_kernel_spmd

BF16 = ml_dtypes.bfloat16
F32 = mybir.dt.float32
BF = mybir.dt.bfloat16

NC = 8            # cores
B = 4             # batch
N = 2048          # sequence
DIM = 1024
NH = 16           # heads total
HD = 64           # head dim
HPC = NH // NC    # heads per core = 2
SCALE = HD ** -0.5
BN = B * N        # 8192 tokens
RPB = N // NC     # rows per (core, batch) after re-shard = 256
KC = DIM // 128   # contraction chunks for projections = 8
NKC = N // 128    # key chunks per batch = 16
CW = HD + 1       # value width incl. ones column = 65


def build(PIPELINE=True, NPHASE=2, MERGEH=False):
    QTAG = 2 if PIPELINE else B
    nc = bacc.Bacc("TRN2", target_bir_lowering=False, debug=False,
                   num_devices=NC)

    xT = nc.dram_tensor("xT", [DIM, BN], BF, kind="ExternalInput")
    wqkT = nc.dram_tensor("wqkT", [DIM, 2 * 128], BF, kind="ExternalInput")
    bqk = nc.dram_tensor("bqk", [2 * 128, 1], F32, kind="ExternalInput")
    ctxa = nc.dram_tensor("ctxa", [B, HPC, 128, NKC * CW], BF,
                          kind="ExternalInput")
    woutT = nc.dram_tensor("woutT", [DIM, DIM], BF, kind="ExternalInput")
    boutb = nc.dram_tensor("boutb", [128, DIM], F32, kind="ExternalInput")
    # out rows: batch-major, 256 rows per batch
    out = nc.dram_tensor("out", [B * RPB, DIM], F32, kind="ExternalOutput")

    # AllToAll bounce buffers, NPHASE collectives each covering B//NPHASE
    # batches; chunk j holds rows [j*256:(j+1)*256] of each covered batch
    bpp = B // NPHASE        # batches per phase
    a2a_in = [nc.dram_tensor(f"a2a_in{p}", [NC, 128, bpp * RPB], BF)
              for p in range(NPHASE)]
    a2a_out = [nc.dram_tensor(f"a2a_out{p}", [NC, 128, bpp * RPB], BF)
               for p in range(NPHASE)]

    rscr = [nc.dram_tensor(f"rscr{i}", [1, 512], F32) for i in range(8)]
    _scr_idx = [0]
    wu_dram = nc.dram_tensor("wu_dram", [128, 512], F32)

    with tile.TileContext(nc) as tc:
        with tc.tile_pool(name="const", bufs=1) as const, \
             tc.tile_pool(name="qk", bufs=1) as qkpool, \
             tc.tile_pool(name="xt", bufs=8 if not PIPELINE else 10) as xtpool, \
             tc.tile_pool(name="pt", bufs=2) as ptpool, \
             tc.tile_pool(name="r1", bufs=4) as r1pool, \
             tc.tile_pool(name="rb", bufs=4) as rbpool, \
             tc.tile_pool(name="ho", bufs=4) as hopool, \
             tc.tile_pool(name="sl", bufs=16) as slpool, \
             tc.tile_pool(name="ob", bufs=4) as obpool, \
             tc.tile_pool(name="pc", bufs=3) as pcpool, \
             tc.tile_pool(name="pss", bufs=2, space="PSUM") as pss_pool, \
             tc.tile_pool(name="psm", bufs=4, space="PSUM") as psm_pool:

            # ---- HAM warm-up: keep the PE busy through the initial
            # input-DMA window so the real matmuls start at 2.4 GHz ----
            wu = hopool.tile([HD, 512], BF, tag="ho", name="wu")
            nc.vector.memset(wu[:], 1.0)
            wu_ps = psm_pool.tile([128, 512], F32, tag="psm", name="wu_ps")
            for i in range(52):
                nc.tensor.matmul(wu_ps[:], wu[:, 0:128], wu[:],
                                 start=(i == 0), stop=(i == 51))
            wu_o = obpool.tile([128, 512], F32, tag="ob", name="wu_o")
            nc.scalar.copy(wu_o[:], wu_ps[:])
            nc.sync.dma_start(out=wu_dram[:], in_=wu_o[:])

            # ---- small constants needed up front ----
            wqk_sb = []
            for kc in range(KC):
                t = const.tile([128, 256], BF, tag=f"wqk{kc}")
                nc.sync.dma_start(out=t[:], in_=wqkT[kc * 128:(kc + 1) * 128, :])
                wqk_sb.append(t)
            bq_sb = []
            for fb in range(2):
                t = const.tile([128, 1], F32, tag=f"bq{fb}")
                nc.sync.dma_start(out=t[:], in_=bqk[fb * 128:(fb + 1) * 128, :])
                bq_sb.append(t)

            wout_sb = []
            bout_sb = const.tile([128, DIM], F32, tag="bout")
            ctx_sb = {}
            qk_tiles = {}
            xt_tiles = {}

            def load_out_consts():
                for fc in range(KC):
                    t = const.tile([128, DIM], BF, tag=f"wout{fc}",
                                   name=f"wout{fc}")
                    nc.sync.dma_start(
                        out=t[:], in_=woutT[fc * 128:(fc + 1) * 128, :])
                    wout_sb.append(t)
                nc.sync.dma_start(out=bout_sb[:], in_=boutb[:])

            def load_ctx(b):
                for h in range(HPC):
                    t = const.tile([128, NKC * CW], BF, tag=f"ctx{b}{h}",
                                   name=f"ctx{b}_{h}")
                    nc.sync.dma_start(out=t[:], in_=ctxa[b, h, :, :])
                    ctx_sb[b, h] = t

            def prefetch_x(b):
                """Issue the xT DMAs and allocate q/k tiles for batch b."""
                qT = qkpool.tile([128, N], BF, tag=f"qT{b % QTAG}", name=f"qT{b}")
                kT = qkpool.tile([128, N], BF, tag=f"kT{b % QTAG}", name=f"kT{b}")
                qk_tiles[b] = (qT, kT)
                xts = []
                for kc in range(KC):
                    xt = xtpool.tile([128, N], BF, tag="xt",
                                     name=f"xtb{b}_{kc}")
                    nc.sync.dma_start(
                        out=xt[:], in_=xT[kc * 128:(kc + 1) * 128,
                                          b * N:(b + 1) * N])
                    xts.append(xt)
                xt_tiles[b] = xts

            def qkproj_slice(b, t):
                """Project token chunk t (512 tokens) of batch b."""
                qT, kT = qk_tiles[b]
                xts = xt_tiles[b]
                for fb, dst in ((1, kT), (0, qT)):
                    ps = psm_pool.tile([128, 512], F32, tag="psm",
                                       name=f"psq{b}_{t}_{fb}")
                    for kc in range(KC):
                        nc.tensor.matmul(
                            ps[:], wqk_sb[kc][:, fb * 128:(fb + 1) * 128],
                            xts[kc][:, t * 512:(t + 1) * 512],
                            start=(kc == 0), stop=(kc == KC - 1))
                    nc.vector.tensor_scalar_add(
                        dst[:, t * 512:(t + 1) * 512], ps[:], bq_sb[fb][:])

            def attention_group(b, h, qg):
                """Scores+softmax+values for one (head, 1024-query) group."""
                qT, kT = qk_tiles[b]
                hp = h * HD
                q0 = qg * 1024
                pt = ptpool.tile([128, NKC * 1024], BF, tag="pt",
                                 name=f"pt{b}_{h}_{qg}")
                for kc in range(NKC):
                    ps = pss_pool.tile([128, 1024], F32, tag="pss",
                                       name=f"pss{b}{h}{qg}{kc}")
                    for hf in range(2):
                        nc.tensor.matmul(
                            ps[:, hf * 512:(hf + 1) * 512],
                            kT[hp:hp + HD, kc * 128:(kc + 1) * 128],
                            qT[hp:hp + HD, q0 + hf * 512:q0 + (hf + 1) * 512],
                            start=True, stop=True)
                    nc.scalar.activation(
                        pt[:, kc * 1024:(kc + 1) * 1024], ps[:],
                        mybir.ActivationFunctionType.Exp, scale=SCALE)
                for qc in range(2):  # 512-query chunks
                    pav = psm_pool.tile([CW, 512], F32, tag="psm",
                                        name=f"pav{b}{h}{qg}{qc}")
                    for kc in range(NKC):
                        nc.tensor.matmul(
                            pav[:], ctx_sb[b, h][:, kc * CW:(kc + 1) * CW],
                            pt[:, kc * 1024 + qc * 512:
                               kc * 1024 + (qc + 1) * 512],
                            start=(kc == 0), stop=(kc == NKC - 1))
                    r1 = r1pool.tile([1, 512], F32, tag="r1",
                                     name=f"r1{b}{h}{qg}{qc}")
                    nc.vector.reciprocal(r1[:], pav[HD:CW, :])
                    # broadcast partition 0 -> 64 via a DRAM round-trip so
                    # gpsimd stays free to run collectives asynchronously
                    scr = rscr[_scr_idx[0] % 8]; _scr_idx[0] += 1
                    nc.sync.dma_start(out=scr[:], in_=r1[:])
                    rb = rbpool.tile([HD, 512], F32, tag="rb",
                                     name=f"rb{b}{h}{qg}{qc}")
                    nc.sync.dma_start(out=rb[:],
                                      in_=scr[:].broadcast_to([HD, 512]))
                    ho = hopool.tile([HD, 512], BF, tag="ho",
                                     name=f"ho{b}{h}{qg}{qc}")
                    nc.vector.tensor_tensor(
                        out=ho[:], in0=pav[0:HD, :], in1=rb[:],
                        op=mybir.AluOpType.mult)
                    # queries qq0..qq0+512 span two 256-row chunks
                    qq0 = q0 + qc * 512
                    for half in range(2):
                        j = (qq0 + half * 256) // RPB
                        o = (b % bpp) * RPB
                        nc.sync.dma_start(
                            out=a2a_in[b // bpp][j, h * HD:(h + 1) * HD,
                                                 o:o + RPB],
                            in_=ho[:, half * 256:(half + 1) * 256])

            def attention_pair(b, qg):
                """Both heads' scores+softmax+values for 512 queries.

                The two heads' score matmuls contract over disjoint
                row-groups of the PE array (partitions 0-63 / 64-127) and
                write disjoint PSUM banks, so they run concurrently.
                """
                qT, kT = qk_tiles[b]
                q0 = qg * 512
                pt = ptpool.tile([128, NKC * 1024], BF, tag="pt",
                                 name=f"ptp{b}_{qg}")
                for kc in range(NKC):
                    ps = pss_pool.tile([128, 1024], F32, tag="pss",
                                       name=f"pssp{b}{qg}{kc}")
                    for h in range(HPC):
                        nc.tensor.matmul(
                            ps[:, h * 512:(h + 1) * 512],
                            kT[h * HD:(h + 1) * HD, kc * 128:(kc + 1) * 128],
                            qT[h * HD:(h + 1) * HD, q0:q0 + 512],
                            start=True, stop=True,
                            tile_position=(h * HD, 0))
                    nc.scalar.activation(
                        pt[:, kc * 1024:(kc + 1) * 1024], ps[:],
                        mybir.ActivationFunctionType.Exp, scale=SCALE)
                for h in range(HPC):
                    pav = psm_pool.tile([CW, 512], F32, tag="psm",
                                        name=f"pavp{b}{qg}{h}")
                    for kc in range(NKC):
                        nc.tensor.matmul(
                            pav[:], ctx_sb[b, h][:, kc * CW:(kc + 1) * CW],
                            pt[:, kc * 1024 + h * 512:
                               kc * 1024 + (h + 1) * 512],
                            start=(kc == 0), stop=(kc == NKC - 1))
                    # evict PSUM immediately so the accumulator slot
                    # frees before the (long-latency) normalize chain
                    pc = pcpool.tile([CW, 512], F32, tag="pc",
                                     name=f"pcp{b}{qg}{h}")
                    nc.vector.tensor_copy(pc[:], pav[:])
                    r1 = r1pool.tile([1, 512], F32, tag="r1",
                                     name=f"r1p{b}{qg}{h}")
                    nc.vector.reciprocal(r1[:], pc[HD:CW, :])
                    scr = rscr[_scr_idx[0] % 8]; _scr_idx[0] += 1
                    nc.sync.dma_start(out=scr[:], in_=r1[:])
                    rb = rbpool.tile([HD, 512], F32, tag="rb",
                                     name=f"rbp{b}{qg}{h}")
                    nc.sync.dma_start(out=rb[:],
                                      in_=scr[:].broadcast_to([HD, 512]))
                    ho = hopool.tile([HD, 512], BF, tag="ho",
                                     name=f"hop{b}{qg}{h}")
                    nc.vector.tensor_tensor(
                        out=ho[:], in0=pc[0:HD, :], in1=rb[:],
                        op=mybir.AluOpType.mult)
                    for half in range(2):
                        j = (q0 + half * 256) // RPB
                        o = (b % bpp) * RPB
                        nc.sync.dma_start(
                            out=a2a_in[b // bpp][j, h * HD:(h + 1) * HD,
                                                 o:o + RPB],
                            in_=ho[:, half * 256:(half + 1) * 256])

            def reshard(p):
                nc.gpsimd.collective_compute(
                    "AllToAll", mybir.AluOpType.bypass,
                    replica_groups=[list(range(NC))],
                    ins=[a2a_in[p].ap().opt()], outs=[a2a_out[p].ap().opt()])

            def outproj(b):
                """Output projection for my 256 rows of batch b."""
                p, o = b // bpp, (b % bpp) * RPB
                for rc in range(RPB // 128):
                    sls = []
                    for fc in range(KC):
                        sl = slpool.tile([128, 128], BF, tag="sl",
                                         name=f"sl{b}_{rc}_{fc}")
                        nc.sync.dma_start(
                            out=sl[:],
                            in_=a2a_out[p][fc, :,
                                           o + rc * 128:o + (rc + 1) * 128])
                        sls.append(sl)
                    pso = [psm_pool.tile([128, 512], F32, tag="psm",
                                         name=f"pso{b}_{rc}_{i}")
                           for i in range(2)]
                    for fc in range(KC):
                        for n in range(2):
                            nc.tensor.matmul(
                                pso[n][:], sls[fc][:],
                                wout_sb[fc][:, n * 512:(n + 1) * 512],
                                start=(fc == 0), stop=(fc == KC - 1))
                    for n in range(2):
                        ob = obpool.tile([128, 512], F32, tag="ob",
                                         name=f"ob{b}_{rc}_{n}")
                        nc.vector.tensor_tensor(
                            out=ob[:], in0=pso[n][:],
                            in1=bout_sb[:, n * 512:(n + 1) * 512],
                            op=mybir.AluOpType.add)
                        nc.sync.dma_start(
                            out=out[b * RPB + rc * 128:
                                    b * RPB + (rc + 1) * 128,
                                    n * 512:(n + 1) * 512],
                            in_=ob[:])

            if PIPELINE:
                # software-pipelined emission
                prefetch_x(0)
                load_ctx(0)
                for t in range(4):
                    qkproj_slice(0, t)
                for b in range(B):
                    if b + 1 < B:
                        prefetch_x(b + 1)
                        load_ctx(b + 1)
                    for g, (h, qg) in enumerate(
                            ((0, 0), (0, 1), (1, 0), (1, 1))):
                        attention_group(b, h, qg)
                        if b + 1 < B:
                            qkproj_slice(b + 1, g)
                        elif g == 1:
                            load_out_consts()
                    if (b + 1) % bpp == 0:
                        reshard(b // bpp)
                for b in range(B):
                    outproj(b)
            else:
                # monolithic phases (v1-style), with the qk projection of
                # later batches staggered after earlier batches' attention
                # so the cold-clock ramp only fronts two batches of work
                for b in range(2):
                    prefetch_x(b)
                    load_ctx(b)
                    for t in range(4):
                        qkproj_slice(b, t)
                load_out_consts()
                for b in range(B):
                    if MERGEH:
                        if b + 2 < B:
                            prefetch_x(b + 2)
                            load_ctx(b + 2)
                            for t in range(4):
                                qkproj_slice(b + 2, t)
                        for qg in range(4):
                            attention_pair(b, qg)
                            # fill PE slack in the last batch with the
                            # output projection of the already-resharded
                            # first phase
                        if b == B - 1 and NPHASE > 1:
                            for bb in range(bpp):
                                outproj(bb)
                    else:
                        if b + 2 < B:
                            prefetch_x(b + 2)
                            load_ctx(b + 2)
                            for t in range(4):
                                qkproj_slice(b + 2, t)
                        for h, qg in ((0, 0), (0, 1), (1, 0), (1, 1)):
                            attention_group(b, h, qg)
                    if (b + 1) % bpp == 0:
                        reshard(b // bpp)
                for b in range(bpp if (MERGEH and NPHASE > 1) else 0, B):
                    outproj(b)
    nc.compile()
    return nc


def prep_inputs(x, context, Wqkv, bqkv, Wout, bout):
    """Host-side sharding: returns in_maps for the 8 cores."""
    x = np.asarray(x, np.float32)
    context = np.asarray(context, np.float32)
    Wqkv = np.asarray(Wqkv, np.float32)
    bqkv = np.asarray(bqkv, np.float32)
    Wout = np.asarray(Wout, np.float32)
    bout = np.asarray(bout, np.float32)

    xT = np.ascontiguousarray(x.reshape(BN, DIM).T).astype(BF16)
    woutT = np.ascontiguousarray(Wout.T).astype(BF16)
    boutb = np.broadcast_to(bout, (128, DIM)).astype(np.float32).copy()

    in_maps = []
    for c in range(NC):
        h0 = c * HPC
        # feature order: [q_h0 | q_h1] then [k_h0 | k_h1]
        wq = Wqkv[h0 * HD:(h0 + HPC) * HD]
        wk = Wqkv[DIM + h0 * HD:DIM + (h0 + HPC) * HD]
        wqkT = np.ascontiguousarray(
            np.concatenate([wq, wk], axis=0).T).astype(BF16)
        bq = np.concatenate([bqkv[h0 * HD:(h0 + HPC) * HD],
                             bqkv[DIM + h0 * HD:DIM + (h0 + HPC) * HD]])
        bq = bq.reshape(2 * 128, 1).astype(np.float32)
        ctxa = np.ones((B, HPC, 128, NKC, CW), np.float32)
        for h in range(HPC):
            g = h0 + h
            arr = context[:, :, g * HD:(g + 1) * HD].reshape(B, NKC, 128, HD)
            ctxa[:, h, :, :, :HD] = arr.transpose(0, 2, 1, 3)
        in_maps.append({
            "xT": xT,
            "wqkT": wqkT,
            "bqk": bq,
            "ctxa": ctxa.reshape(B, HPC, 128, NKC * CW).astype(BF16),
            "woutT": woutT,
            "boutb": boutb,
        })
    return in_maps


_NC_CACHE = None


import os


def _get_nc():
    global _NC_CACHE
    if _NC_CACHE is None:
        _NC_CACHE = build(
            PIPELINE=os.environ.get("K_PIPELINE", "0") == "1",
            NPHASE=int(os.environ.get("K_NPHASE", "2")),
            MERGEH=os.environ.get("K_MERGEH", "1") == "1")
    return _NC_CACHE


def run(in_maps, trace=False):
    nc = _get_nc()
    res = run_bass_kernel_spmd(nc, in_maps, core_ids=list(range(NC)),
                               trace=trace)
    # core c's out = [B*256, DIM]: rows [c*256:(c+1)*256] of each batch
    full = np.empty((B, N, DIM), np.float32)
    for c in range(NC):
        o = np.asarray(res.results[c]["out"]).reshape(B, RPB, DIM)
        full[:, c * RPB:(c + 1) * RPB, :] = o
    return full, res


def kernel(x, context, Wqkv, bqkv, Wout, bout):
    in_maps = prep_inputs(x, context, Wqkv, bqkv, Wout, bout)
    out, _ = run(in_maps, trace=False)
    return out
